# revision 1
# baseline (speedup 1.0000x reference)
"""GQA (grouped-query attention) Trainium2 kernel, 8-core SPMD.

Sharding: TP=4 over kv-heads x DP=2 over batch  (core = b*4 + g).
Each core computes, for its batch b and kv-head g (q-heads 4g..4g+3):
  QKV projections -> RoPE -> causal softmax(QK^T)V -> partial x@Wo
entirely in transposed layout (feature dim on SBUF partitions), then the
host sums the 4 partial Wo outputs per batch (the TP all-reduce).

Dataflow notes:
 - all big matmuls use fp32r (full PE rate at N>=256, ~tf32 precision)
 - probabilities P^T kept in bf16 (consistent numerator/denominator)
 - softmax runs in S^T[k,q] orientation: denominators via a ones-row
   matmul accumulated on PSUM alongside the P^T@V accumulation
 - no max-subtraction: scores are bounded (~+-5) for this problem size
 - causal structure: strictly-upper k-blocks skipped, diagonal blocks
   masked multiplicatively after exp
"""

import math
import sys

import numpy as np

if "/opt/trn_rl_repo" not in sys.path:
    sys.path.insert(0, "/opt/trn_rl_repo")

import ml_dtypes

B, S, D = 2, 2048, 2048
HQ, HKV, DH = 16, 4, 128
G = HQ // HKV            # q-heads per kv-head = 4
NCORES = 8
ROPE_THETA = 10000.0
SCALE = 1.0 / math.sqrt(DH)

SB = 512                 # wide column block (moving operand)
NSB = S // SB            # 4
ND = D // 128            # 16 contraction tiles
NKB = S // 128           # 16 key blocks

_CACHE = {}


def _build_nc():
    import concourse.bass as bass
    import concourse.mybir as mybir
    import concourse.tile as tile
    from concourse import bacc
    from concourse.masks import make_identity

    f32 = mybir.dt.float32
    bf16 = mybir.dt.bfloat16
    f32r = mybir.dt.float32r
    AF = mybir.ActivationFunctionType

    nc = bacc.Bacc(
        trn_type="TRN2", target_bir_lowering=False, debug=False,
        num_devices=NCORES,
    )

    xt_d = nc.dram_tensor("xt", [D, S], f32r, kind="ExternalInput").ap()
    wqt_d = nc.dram_tensor("wqt", [D, G * DH], f32r, kind="ExternalInput").ap()
    wkt_d = nc.dram_tensor("wkt", [D, DH], f32r, kind="ExternalInput").ap()
    wvt_d = nc.dram_tensor("wvt", [D, DH], f32r, kind="ExternalInput").ap()
    wot_d = nc.dram_tensor("wot", [G * DH, D], f32r, kind="ExternalInput").ap()
    cos_d = nc.dram_tensor("cost", [DH, S], f32, kind="ExternalInput").ap()
    sin_d = nc.dram_tensor("sints", [DH, S], f32, kind="ExternalInput").ap()
    msk_d = nc.dram_tensor("masks", [G, 128, SB], bf16, kind="ExternalInput").ap()
    onc_d = nc.dram_tensor("onesc", [128, 1], bf16, kind="ExternalInput").ap()
    onr_d = nc.dram_tensor("onesr", [1, 128], f32r, kind="ExternalInput").ap()
    y_d = nc.dram_tensor("y", [S, D], f32, kind="ExternalOutput").ap()

    from contextlib import ExitStack

    with tile.TileContext(nc) as tc, ExitStack() as stack, \
            nc.allow_low_precision(reason="f32r matmul operands (~tf32)"):
        # ---- pools that live for (almost) the whole kernel ----
        persist = stack.enter_context(tc.tile_pool(name="persist", bufs=1))

        qrt = [persist.tile([128, S], f32r, name=f"qrt{h}", tag=f"qrt{h}") for h in range(G)]
        krt = persist.tile([128, S], f32r, name="krt", tag="krt")
        vsb = [persist.tile([128, DH], bf16, name=f"v{k}", tag=f"v{k}") for k in range(NKB)]
        masks = [persist.tile([128, SB], bf16, name=f"msk{j}", tag=f"msk{j}") for j in range(G)]
        ident = persist.tile([128, 128], f32, name="ident", tag="ident")
        ones_col = persist.tile([128, 1], bf16, name="ones_col", tag="ones_col")
        ones_row = persist.tile([1, 128], f32r, name="ones_row", tag="ones_row")

        make_identity(nc, ident[:])
        nc.sync.dma_start(ones_col[:], onc_d[:])
        nc.sync.dma_start(ones_row[:], onr_d[:])
        for j in range(G):
            nc.sync.dma_start(masks[j][:], msk_d[j])

        # =========== phase 1: projections + RoPE ===========
        with tc.tile_pool(name="ph1w", bufs=1) as ph1w, \
             tc.tile_pool(name="xtp", bufs=24) as xtp, \
             tc.tile_pool(name="rope", bufs=4) as rope, \
             tc.tile_pool(name="vtsb", bufs=2) as vtsb, \
             tc.tile_pool(name="proj_ps", bufs=6, space="PSUM") as proj_ps, \
             tc.tile_pool(name="vtr_ps", bufs=2, space="PSUM") as vtr_ps:

            cost = ph1w.tile([128, S], f32, name="cost", tag="cost")
            sint = ph1w.tile([128, S], f32, name="sint", tag="sint")
            nc.sync.dma_start(cost[:], cos_d[:])
            nc.sync.dma_start(sint[:], sin_d[:])

            wqt_t = [ph1w.tile([128, G * DH], f32r, name=f"wq{i}", tag=f"wq{i}") for i in range(ND)]
            wkt_t = [ph1w.tile([128, DH], f32r, name=f"wk{i}", tag=f"wk{i}") for i in range(ND)]
            wvt_t = [ph1w.tile([128, DH], f32r, name=f"wv{i}", tag=f"wv{i}") for i in range(ND)]
            for i in range(ND):
                nc.sync.dma_start(wqt_t[i][:], wqt_d[128 * i:128 * (i + 1), :])
                nc.sync.dma_start(wkt_t[i][:], wkt_d[128 * i:128 * (i + 1), :])
                nc.sync.dma_start(wvt_t[i][:], wvt_d[128 * i:128 * (i + 1), :])

            def rope_evict(ps, out_slice, c0):
                ts_ = rope.tile([128, SB], f32, name="tsin", tag="tsin")
                tcs = rope.tile([128, SB], f32, name="tcos", tag="tcos")
                cs = slice(c0, c0 + SB)
                nc.vector.tensor_mul(ts_[0:64, :], ps[64:128, :], sint[0:64, cs])
                nc.vector.tensor_mul(ts_[64:128, :], ps[0:64, :], sint[64:128, cs])
                nc.vector.tensor_mul(tcs[:], ps[:], cost[:, cs])
                nc.vector.tensor_add(out_slice, tcs[:], ts_[:])

            for sb in range(NSB):
                c0 = SB * sb
                xt_t = []
                for i in range(ND):
                    t = xtp.tile([128, SB], f32r, name="xt", tag="xt")
                    nc.sync.dma_start(
                        t[:], xt_d[128 * i:128 * (i + 1), c0:c0 + SB])
                    xt_t.append(t)

                for qh in range(G):
                    ps = proj_ps.tile([128, SB], f32, name="pp", tag="pp")
                    for i in range(ND):
                        nc.tensor.matmul(
                            ps[:], wqt_t[i][:, 128 * qh:128 * (qh + 1)],
                            xt_t[i][:], start=(i == 0), stop=(i == ND - 1))
                    rope_evict(ps, qrt[qh][:, c0:c0 + SB], c0)

                ps = proj_ps.tile([128, SB], f32, name="pp", tag="pp")
                for i in range(ND):
                    nc.tensor.matmul(ps[:], wkt_t[i][:], xt_t[i][:],
                                     start=(i == 0), stop=(i == ND - 1))
                rope_evict(ps, krt[:, c0:c0 + SB], c0)

                # V^T then PE-transpose into [k,dv] bf16 tiles
                ps = proj_ps.tile([128, SB], f32, name="pp", tag="pp")
                for i in range(ND):
                    nc.tensor.matmul(ps[:], wvt_t[i][:], xt_t[i][:],
                                     start=(i == 0), stop=(i == ND - 1))
                vt_sb = vtsb.tile([128, SB], f32, name="vt", tag="vt")
                nc.scalar.copy(vt_sb[:], ps[:])
                for ks in range(SB // 128):
                    vp = vtr_ps.tile([128, 128], f32, name="vp", tag="vp")
                    nc.tensor.transpose(
                        vp[:], vt_sb[:, 128 * ks:128 * (ks + 1)], ident[:])
                    nc.scalar.copy(vsb[4 * sb + ks][:], vp[:])

        # =========== phase 2: attention ===========
        asb = stack.enter_context(tc.tile_pool(name="asb", bufs=1))
        a_t = [asb.tile([128, S], f32r, name=f"a{h}", tag=f"a{h}") for h in range(G)]

        with tc.tile_pool(name="psb", bufs=6) as psb, \
             tc.tile_pool(name="small", bufs=4) as small, \
             tc.tile_pool(name="s_ps", bufs=2, space="PSUM") as s_ps, \
             tc.tile_pool(name="a_ps", bufs=2, space="PSUM") as a_ps, \
             tc.tile_pool(name="d_ps", bufs=2, space="PSUM") as d_ps, \
             tc.tile_pool(name="b_ps", bufs=2, space="PSUM") as b_ps:

            def attn_block(h, qb):
                """scores -> exp -> (mask) -> PV & ones accumulation"""
                q0 = SB * qb
                nkb = (q0 + SB) // 128
                aps = a_ps.tile([128, SB], f32, name="aps", tag="aps")
                dps = d_ps.tile([1, SB], f32, name="dps", tag="dps")
                for kb in range(nkb):
                    sps = s_ps.tile([128, SB], f32, name="sps", tag="sps")
                    nc.tensor.matmul(
                        sps[:], krt[:, 128 * kb:128 * (kb + 1)],
                        qrt[h][:, q0:q0 + SB],
                        start=True, stop=True, skip_group_check=True)
                    p = psb.tile([128, SB], bf16, name="p", tag="p")
                    nc.scalar.activation(p[:], sps[:], AF.Exp, scale=SCALE)
                    j = kb - 4 * qb
                    if j >= 0:
                        nc.vector.tensor_mul(p[:], p[:], masks[j][:])
                    nc.tensor.matmul(
                        aps[:], vsb[kb][:], p[:],
                        start=(kb == 0), stop=(kb == nkb - 1),
                        skip_group_check=True)
                    nc.tensor.matmul(
                        dps[:], ones_col[:], p[:],
                        start=(kb == 0), stop=(kb == nkb - 1),
                        skip_group_check=True)
                return aps, dps

            def attn_finalize(h, qb, aps, dps):
                """1/denominator -> broadcast over partitions -> normalize"""
                q0 = SB * qb
                den = small.tile([1, SB], f32, name="den", tag="den")
                nc.vector.tensor_copy(den[:], dps[:])
                rec = small.tile([1, SB], f32r, name="rec", tag="rec")
                nc.vector.reciprocal(rec[:], den[:])
                bps = b_ps.tile([128, SB], f32, name="bps", tag="bps")
                nc.tensor.matmul(bps[:], ones_row[:], rec[:],
                                 start=True, stop=True, skip_group_check=True)
                rbc = small.tile([128, SB], f32, name="rbc", tag="rbc")
                nc.scalar.copy(rbc[:], bps[:])
                nc.vector.tensor_mul(a_t[h][:, q0:q0 + SB], aps[:], rbc[:])

            # software-pipelined: finalize (h,qb) after next block's scores
            pend = None
            for h in range(G):
                for qb in range(NSB):
                    cur = (h, qb, *attn_block(h, qb))
                    if pend is not None:
                        attn_finalize(*pend)
                    pend = cur
            attn_finalize(*pend)

        # =========== phase 3: partial Wo projection ===========
        with tc.tile_pool(name="ph3w", bufs=1) as ph3w, \
             tc.tile_pool(name="ysb", bufs=4) as ysb, \
             tc.tile_pool(name="y_ps", bufs=6, space="PSUM") as y_ps:
            wot_t = [ph3w.tile([128, D], f32r, name=f"wo{h}", tag=f"wo{h}") for h in range(G)]
            for h in range(G):
                nc.sync.dma_start(wot_t[h][:], wot_d[128 * h:128 * (h + 1), :])
            for sb in range(NKB):
                for eb in range(NSB):
                    yp = y_ps.tile([128, SB], f32, name="yp", tag="yp")
                    for h in range(G):
                        nc.tensor.matmul(
                            yp[:], a_t[h][:, 128 * sb:128 * (sb + 1)],
                            wot_t[h][:, SB * eb:SB * (eb + 1)],
                            start=(h == 0), stop=(h == G - 1))
                    yt = ysb.tile([128, SB], f32, name="yt", tag="yt")
                    nc.vector.tensor_copy(yt[:], yp[:])
                    nc.sync.dma_start(
                        y_d[128 * sb:128 * (sb + 1), SB * eb:SB * (eb + 1)],
                        yt[:])

    nc.compile()
    return nc


def _rope_tables():
    inv = 1.0 / (ROPE_THETA ** (np.arange(0, DH, 2, dtype=np.float64) / DH))
    pos = np.arange(S, dtype=np.float64)
    theta = np.concatenate([np.outer(pos, inv)] * 2, axis=1)  # [S, DH]
    cosT = np.cos(theta).T.astype(np.float32)                 # [DH, S]
    sinT = np.sin(theta).T.astype(np.float32)
    sints = np.concatenate([-sinT[:64], sinT[64:]], axis=0)
    return np.ascontiguousarray(cosT), np.ascontiguousarray(sints)


def _mask_tiles():
    r_ = np.arange(128)[:, None]
    c = np.arange(SB)[None, :]
    m = np.stack([(c >= 128 * j + r_) for j in range(G)]).astype(np.float32)
    return m.astype(ml_dtypes.bfloat16)


def build_in_maps(x, Wq, Wk, Wv, Wo):
    x = np.asarray(x, np.float32)
    Wq = np.asarray(Wq, np.float32)
    Wk = np.asarray(Wk, np.float32)
    Wv = np.asarray(Wv, np.float32)
    Wo = np.asarray(Wo, np.float32)
    cosT, sints = _rope_tables()
    masks = _mask_tiles()
    in_maps = []
    for core in range(NCORES):
        b, g = divmod(core, HKV)
        in_maps.append({
            "xt": np.ascontiguousarray(x[b].T),
            "wqt": np.ascontiguousarray(Wq[G * DH * g:G * DH * (g + 1)].T),
            "wkt": np.ascontiguousarray(Wk[DH * g:DH * (g + 1)].T),
            "wvt": np.ascontiguousarray(Wv[DH * g:DH * (g + 1)].T),
            "wot": np.ascontiguousarray(Wo[:, G * DH * g:G * DH * (g + 1)].T),
            "cost": cosT,
            "sints": sints,
            "masks": masks,
            "onesc": np.ones((128, 1), ml_dtypes.bfloat16),
            "onesr": np.ones((1, 128), np.float32),
        })
    return in_maps


def get_nc():
    if "nc" not in _CACHE:
        _CACHE["nc"] = _build_nc()
    return _CACHE["nc"]


def kernel(x, Wq, Wk, Wv, Wo):
    from concourse.bass_utils import run_bass_kernel_spmd

    nc = get_nc()
    in_maps = build_in_maps(x, Wq, Wk, Wv, Wo)
    res = run_bass_kernel_spmd(nc, in_maps, list(range(NCORES)))
    parts = [res.results[c]["y"] for c in range(NCORES)]
    y = np.stack([
        parts[0] + parts[1] + parts[2] + parts[3],
        parts[4] + parts[5] + parts[6] + parts[7],
    ]).astype(np.float32)
    return y



# revision 44
# speedup vs baseline: 2.2195x; 2.2195x over previous
"""GQA (grouped-query attention) Trainium2 kernel, 8-core SPMD.

Sharding: TP=4 over kv-heads x DP=2 over batch  (core = b*4 + g).
Each core computes, for its batch b and kv-head g (q-heads 4g..4g+3):
  QKV projections -> RoPE -> causal softmax(QK^T)V -> partial x@Wo
entirely in transposed layout (feature dim on SBUF partitions), then the
host sums the 4 partial Wo outputs per batch (the TP all-reduce).

Single fused pipeline over the 4 query/sequence blocks of 512:
  proj(sb) -> attention(qb=sb) -> Wo-chunk(qb-1) quarters interleaved
between attention heads so the PE never drains.

Dataflow notes:
 - all matmul operands in bf16 (full PE rate, halves HBM traffic)
 - causal structure: strictly-upper k-blocks skipped; the 4 diagonal
   k-blocks use narrowed moving operands (512/384/256/128 queries) and a
   multiplicative triangular mask after exp
 - softmax runs in S^T[k,q] orientation; denominators: exp tiles are
   accumulated on the vector engine (bf16 2x) into dacc, then a single
   ones-column matmul per (head, q-block) reduces partitions on the PE
 - no max-subtraction: scores are bounded (~+-5) for this problem size
 - V^T -> V[k,dv] reorientation via DMA transpose (16-bit), not the PE
"""

import math
import sys

import numpy as np

if "/opt/trn_rl_repo" not in sys.path:
    sys.path.insert(0, "/opt/trn_rl_repo")

import ml_dtypes

B, S, D = 2, 2048, 2048
HQ, HKV, DH = 16, 4, 128
G = HQ // HKV            # q-heads per kv-head = 4
NCORES = 8
ROPE_THETA = 10000.0
SCALE = 1.0 / math.sqrt(DH)

SB = 512                 # wide column block (moving operand)
NSB = S // SB            # 4
ND = D // 128            # 16 contraction tiles
NKB = S // 128           # 16 key blocks

_CACHE = {}


def _build_nc():
    import concourse.bass as bass
    import concourse.mybir as mybir
    import concourse.tile as tile
    from concourse import bacc

    f32 = mybir.dt.float32
    bf16 = mybir.dt.bfloat16
    f32r = mybir.dt.float32r
    AF = mybir.ActivationFunctionType

    nc = bacc.Bacc(
        trn_type="TRN2", target_bir_lowering=False, debug=False,
        num_devices=NCORES,
    )

    xt_d = nc.dram_tensor("xt", [D, S], bf16, kind="ExternalInput").ap()
    wqkv_d = nc.dram_tensor("wqkv", [D, (G + 2) * DH], bf16, kind="ExternalInput").ap()
    wot_d = nc.dram_tensor("wot", [G * DH, D], bf16, kind="ExternalInput").ap()
    cos_d = nc.dram_tensor("cost", [DH, S], f32, kind="ExternalInput").ap()
    sin_d = nc.dram_tensor("sints", [DH, S], f32, kind="ExternalInput").ap()
    msk_d = nc.dram_tensor("masks", [128, SB], bf16, kind="ExternalInput").ap()
    y_d = nc.dram_tensor("y", [S, D], bf16, kind="ExternalOutput").ap()

    from contextlib import ExitStack

    with tile.TileContext(nc) as tc, ExitStack() as stack, \
            nc.allow_low_precision(reason="bf16 matmul operands"):
        persist = stack.enter_context(tc.tile_pool(name="persist", bufs=1))

        # persistent SBUF tensors (per-block tiles: no reliance on sub-tile
        # dependency tracking)
        xts = [[persist.tile([128, SB], bf16, name=f"x{s}_{i}", tag=f"x{s}_{i}")
                for i in range(ND)] for s in range(NSB)]
        wqkv = [persist.tile([128, (G + 2) * DH], bf16, name=f"w{i}", tag=f"w{i}")
                for i in range(ND)]
        wot = [persist.tile([128, D], bf16, name=f"wo{h}", tag=f"wo{h}")
               for h in range(G)]
        qrt = [[persist.tile([128, SB], bf16, name=f"q{h}_{s}", tag=f"q{h}_{s}")
                for s in range(NSB)] for h in range(G)]
        krt = [persist.tile([128, SB], bf16, name=f"k{s}", tag=f"k{s}")
               for s in range(NSB)]
        vsb = [persist.tile([128, DH], bf16, name=f"v{k}", tag=f"v{k}")
               for k in range(NKB)]
        a_t = [[persist.tile([128, SB], bf16, name=f"a{h}_{s}", tag=f"a{h}_{s}")
                for s in range(NSB)] for h in range(G)]
        cost = [persist.tile([128, SB], f32, name=f"cost{s}", tag=f"cost{s}")
                for s in range(NSB)]
        sint = [persist.tile([128, SB], f32, name=f"sint{s}", tag=f"sint{s}")
                for s in range(NSB)]
        mask = persist.tile([128, SB], bf16, name="mask", tag="mask")

        # working rings (SBUF)
        p_pool = stack.enter_context(tc.tile_pool(name="pp", bufs=4))
        dacc_pool = stack.enter_context(tc.tile_pool(name="dap", bufs=2))
        vsbw_pool = stack.enter_context(tc.tile_pool(name="vsw", bufs=2))
        rope_pool = stack.enter_context(tc.tile_pool(name="rope", bufs=2))
        fin_pool = stack.enter_context(tc.tile_pool(name="fin", bufs=2))
        yt_pool = stack.enter_context(tc.tile_pool(name="yt", bufs=2))

        # PSUM: mm(3) shared by proj/Wo + sps(3) + aps(2) = 8 banks
        mm_ps = stack.enter_context(tc.tile_pool(name="mm_ps", bufs=3, space="PSUM"))
        s_ps = stack.enter_context(tc.tile_pool(name="s_ps", bufs=3, space="PSUM"))
        a_ps = stack.enter_context(tc.tile_pool(name="a_ps", bufs=2, space="PSUM"))

        # weights + first x block, interleaved so matmul i can start early
        for i in range(ND):
            nc.sync.dma_start(wqkv[i][:], wqkv_d[128 * i:128 * (i + 1), :])
            nc.sync.dma_start(xts[0][i][:], xt_d[128 * i:128 * (i + 1), 0:SB])
        # first rope-table chunk right behind (gates the first rope_evict);
        # the rest follows in proj_block(0)
        nc.sync.dma_start(cost[0][:], cos_d[:, 0:SB])
        nc.sync.dma_start(sint[0][:], sin_d[:, 0:SB])
        # small tensors: needed only from attention(0) on
        nc.sync.dma_start(mask[:], msk_d[:])

        def rope_evict(ps, out_tile, sb):
            # Cross-half (rotate-half) reads live on the PSUM operand: the
            # BIR verifier only requires equal base partitions when BOTH
            # tensor_tensor inputs are in SBUF. Muls read PSUM -> DVE; the
            # final add is SBUF-only and partition-aligned -> Pool engine.
            ts_ = rope_pool.tile([128, SB], f32, name="tsin", tag="tsin")
            tcs = rope_pool.tile([128, SB], f32, name="tcos", tag="tcos")
            nc.vector.tensor_mul(ts_[0:64, :], ps[64:128, :], sint[sb][0:64, :])
            nc.vector.tensor_mul(ts_[64:128, :], ps[0:64, :], sint[sb][64:128, :])
            nc.vector.tensor_mul(tcs[:], ps[:], cost[sb][:, :])
            nc.gpsimd.tensor_add(out_tile[:], tcs[:], ts_[:])

        def proj_block(sb):
            c0 = SB * sb
            xt = xts[sb]
            KO, VO = G * DH, (G + 1) * DH  # column offsets of Wk / Wv

            def mm_group(lo, interleaved=()):
                ps = mm_ps.tile([128, SB], f32, name="pp", tag="pp")
                for i in range(ND):
                    nc.tensor.matmul(
                        ps[:], wqkv[i][:, lo:lo + DH], xt[i][:],
                        start=(i == 0), stop=(i == ND - 1),
                        skip_group_check=bool(interleaved))
                return ps

            if sb == 0:
                # x0 tiles arrive at DMA pace: interleave all 6 psum groups
                # by contraction index so the PE rides the DMA wave,
                # borrowing the (not yet used) attention PSUM banks
                pools = [mm_ps, mm_ps, mm_ps, s_ps, s_ps, a_ps]
                tags = ["pp", "pp", "pp", "sps", "sps", "aps"]
                offsets = [VO, KO, 0, DH, 2 * DH, 3 * DH]
                pss = [pool.tile([128, SB], f32, name="pp", tag=t)
                       for pool, t in zip(pools, tags)]
                for i in range(ND):
                    for ps, lo in zip(pss, offsets):
                        nc.tensor.matmul(
                            ps[:], wqkv[i][:, lo:lo + DH], xt[i][:],
                            start=(i == 0), stop=(i == ND - 1),
                            skip_group_check=True)
                vps, kps, qps0, qps1, qps2, qps3 = pss
                rope_evict(kps, krt[sb], sb)
                v_sb = vsbw_pool.tile([128, SB], bf16, name="vsb", tag="vsb")
                nc.scalar.copy(v_sb[:], vps[:])
                rope_evict(qps0, qrt[0][sb], sb)
                rope_evict(qps1, qrt[1][sb], sb)
                rope_evict(qps2, qrt[2][sb], sb)
                rope_evict(qps3, qrt[3][sb], sb)
            else:
                vps = mm_group(VO)
                kps = mm_group(KO)
                v_sb = vsbw_pool.tile([128, SB], bf16, name="vsb", tag="vsb")
                nc.scalar.copy(v_sb[:], vps[:])
                qps0 = mm_group(0)
                rope_evict(kps, krt[sb], sb)
                qps1 = mm_group(DH)
                rope_evict(qps0, qrt[0][sb], sb)
                qps2 = mm_group(2 * DH)
                rope_evict(qps1, qrt[1][sb], sb)
                qps3 = mm_group(3 * DH)
                rope_evict(qps2, qrt[2][sb], sb)
                rope_evict(qps3, qrt[3][sb], sb)

            # DMA schedule: V transpose, tables/weights for upcoming phases,
            # next x block (order = consumption order on the sync queue)
            for c in range(SB // 128):
                nc.sync.dma_start_transpose(
                    vsb[4 * sb + c][:], v_sb[:, 128 * c:128 * (c + 1)])
            if sb + 1 < NSB:
                for i in range(ND):
                    nc.sync.dma_start(
                        xts[sb + 1][i][:],
                        xt_d[128 * i:128 * (i + 1), SB * (sb + 1):SB * (sb + 2)])
            if sb == 0:
                for s in range(1, NSB):
                    nc.sync.dma_start(cost[s][:], cos_d[:, SB * s:SB * (s + 1)])
                    nc.sync.dma_start(sint[s][:], sin_d[:, SB * s:SB * (s + 1)])
            if sb == 1:
                for h in range(G):
                    nc.sync.dma_start(wot[h][:], wot_d[128 * h:128 * (h + 1), :])

        # deferred finalize steps, drained one per attention block
        fin_steps = []

        def drain_one():
            if fin_steps:
                step = fin_steps.pop(0)
                if step is not None:
                    step()

        def drain_all():
            while fin_steps:
                step = fin_steps.pop(0)
                if step is not None:
                    step()

        def finalize_lazy(h, qb, aps, dacc, daccp):
            """Denominator partition-reduce (Pool) -> reciprocal -> normalize,
            as deferred steps.

            Each step is drained with spacing so cross-engine latency
            (DVE->Pool->DVE) never stalls the PE. partition_all_reduce leaves
            the sum on ALL partitions, so no broadcast step is needed.
            """
            from concourse import bass_isa
            st = {}

            def s0():
                nc.gpsimd.tensor_add(daccp[:], daccp[:], dacc[:])

            def s1():
                dall = fin_pool.tile([128, SB], f32, name="dall", tag="dall")
                nc.gpsimd.partition_all_reduce(
                    dall[:], daccp[:], channels=128,
                    reduce_op=bass_isa.ReduceOp.add)
                st["dall"] = dall

            def s2():
                rbc = fin_pool.tile([128, SB], f32, name="rbc", tag="rbc")
                nc.vector.reciprocal(rbc[:], st["dall"][:])
                st["rbc"] = rbc

            def s3():
                nc.vector.tensor_mul(a_t[h][qb][:], aps[:], st["rbc"][:])

            fin_steps.extend([s0, s1, None, s2, None, s3])

        def attn_head(h, qb, defer_finalize=False):
            """scores -> exp -> (mask) -> dacc accumulate -> PV accumulate.

            PE emission has one-block lookahead: scores(kb+1) before PV(kb).
            """
            nkb = 4 * qb + 4
            aps = a_ps.tile([128, SB], f32, name="aps", tag="aps")
            # two partial denominator accumulators: even k-blocks on DVE,
            # odd on Pool (both SBUF-only, legal for GPSIMD); combined at
            # the end on Pool
            dacc = dacc_pool.tile([128, SB], bf16, name="dacc", tag="dacc")
            daccp = dacc_pool.tile([128, SB], bf16, name="daccp", tag="daccp")
            pend = []  # (kb, p, qoff, w), lookahead-2 queue
            for kb in range(nkb):
                j = kb - 4 * qb
                qoff = 128 * j if j > 0 else 0
                w = SB - qoff
                sps = s_ps.tile([128, SB], f32, name="sps", tag="sps")
                nc.tensor.matmul(
                    sps[:, 0:w], krt[kb // 4][:, 128 * (kb % 4):128 * (kb % 4 + 1)],
                    qrt[h][qb][:, qoff:SB],
                    start=True, stop=True, skip_group_check=True)
                p = p_pool.tile([128, SB], bf16, name="p", tag="p")
                nc.scalar.activation(p[:, 0:w], sps[:, 0:w], AF.Exp, scale=SCALE)
                if j >= 0:
                    nc.vector.tensor_mul(p[:, 0:w], p[:, 0:w], mask[:, 0:w])
                eng, acc = (nc.vector, dacc) if kb % 2 == 0 else (nc.gpsimd, daccp)
                if kb < 2:
                    if qoff:
                        eng.memset(acc[:, 0:qoff], 0.0)
                    eng.tensor_copy(acc[:, qoff:SB], p[:, 0:w])
                else:
                    eng.tensor_add(acc[:, qoff:SB], acc[:, qoff:SB], p[:, 0:w])
                if len(pend) == 2:
                    pkb, pp, pqoff, pw = pend.pop(0)
                    nc.tensor.matmul(
                        aps[:, pqoff:SB], vsb[pkb][:], pp[:, 0:pw],
                        start=(pkb == 0), stop=False, skip_group_check=True)
                pend.append((kb, p, qoff, w))
                drain_one()
            while pend:
                pkb, pp, pqoff, pw = pend.pop(0)
                nc.tensor.matmul(
                    aps[:, pqoff:SB], vsb[pkb][:], pp[:, 0:pw],
                    start=(pkb == 0), stop=(not pend), skip_group_check=True)
            if defer_finalize:
                return aps, dacc, daccp
            finalize_lazy(h, qb, aps, dacc, daccp)
            return None

        def wo_quarter(qb, c, final=False):
            """y rows [128c'..] for query block qb, quarter c: 4 eb psums.

            Evictions go to the Pool engine (spare capacity); the final
            quarter evicts on Act and DMAs per-eb to shorten the tail.
            """
            yt = yt_pool.tile([128, D], bf16, name="yt", tag="yt")
            sb128 = 4 * qb + c
            for eb in range(NSB):
                yp = mm_ps.tile([128, SB], f32, name="pp", tag="pp")
                for h in range(G):
                    nc.tensor.matmul(
                        yp[:], a_t[h][qb][:, 128 * c:128 * (c + 1)],
                        wot[h][:, SB * eb:SB * (eb + 1)],
                        start=(h == 0), stop=(h == G - 1))
                if final:
                    # per-eb evict+DMA pipeline to shorten the tail
                    nc.scalar.copy(yt[:, SB * eb:SB * (eb + 1)], yp[:])
                    nc.sync.dma_start(
                        y_d[128 * sb128:128 * (sb128 + 1),
                            SB * eb:SB * (eb + 1)],
                        yt[:, SB * eb:SB * (eb + 1)])
                elif eb % 2 == 0:
                    nc.scalar.copy(yt[:, SB * eb:SB * (eb + 1)], yp[:])
                else:
                    nc.vector.tensor_copy(yt[:, SB * eb:SB * (eb + 1)], yp[:])
            if not final:
                nc.sync.dma_start(
                    y_d[128 * sb128:128 * (sb128 + 1), :], yt[:])

        last = None
        for sb in range(NSB):
            proj_block(sb)
            drain_all()  # a_t[*][sb-1] must be written before wo_quarter reads
            for h in range(G):
                if sb > 0:
                    wo_quarter(sb - 1, h)
                last = attn_head(h, sb,
                                 defer_finalize=(sb == NSB - 1 and h == G - 1))
        drain_all()

        # last head's finalize is latency-critical (it gates the final Wo
        # chunk): run it at 128-column granularity, pipelined against the
        # final Wo quarters
        from concourse import bass_isa
        aps, dacc, daccp = last
        qb = NSB - 1
        dall = fin_pool.tile([128, SB], f32, name="dall", tag="dall")
        for c in range(G):
            cs = slice(128 * c, 128 * (c + 1))
            nc.vector.tensor_add(daccp[:, cs], daccp[:, cs], dacc[:, cs])
            nc.gpsimd.partition_all_reduce(
                dall[:, cs], daccp[:, cs], channels=128,
                reduce_op=bass_isa.ReduceOp.add)
            rbc = fin_pool.tile([128, 128], f32, name="rbcc", tag="rbcc")
            nc.vector.reciprocal(rbc[:], dall[:, cs])
            nc.vector.tensor_mul(a_t[G - 1][qb][:, cs], aps[:, cs], rbc[:])
            wo_quarter(qb, c, final=(c == G - 1))

    nc.compile()
    return nc


def _rope_tables():
    inv = 1.0 / (ROPE_THETA ** (np.arange(0, DH, 2, dtype=np.float64) / DH))
    pos = np.arange(S, dtype=np.float64)
    theta = np.concatenate([np.outer(pos, inv)] * 2, axis=1)  # [S, DH]
    cosT = np.cos(theta).T.astype(np.float32)                 # [DH, S]
    sinT = np.sin(theta).T.astype(np.float32)
    sints = np.concatenate([-sinT[:64], sinT[64:]], axis=0)
    return np.ascontiguousarray(cosT), np.ascontiguousarray(sints)


def build_in_maps(x, Wq, Wk, Wv, Wo):
    bf16 = ml_dtypes.bfloat16
    x = np.asarray(x, np.float32)
    Wq = np.asarray(Wq, np.float32)
    Wk = np.asarray(Wk, np.float32)
    Wv = np.asarray(Wv, np.float32)
    Wo = np.asarray(Wo, np.float32)
    cosT, sints = _rope_tables()
    r_ = np.arange(128)[:, None]
    c_ = np.arange(SB)[None, :]
    mask = (c_ >= r_).astype(np.float32).astype(bf16)
    xt_b = [np.ascontiguousarray(x[b].T.astype(bf16)) for b in range(B)]
    in_maps = []
    for core in range(NCORES):
        b, g = divmod(core, HKV)
        wqkv = np.concatenate([
            Wq[G * DH * g:G * DH * (g + 1)].T,
            Wk[DH * g:DH * (g + 1)].T,
            Wv[DH * g:DH * (g + 1)].T,
        ], axis=1).astype(bf16)
        in_maps.append({
            "xt": xt_b[b],
            "wqkv": np.ascontiguousarray(wqkv),
            "wot": np.ascontiguousarray(
                Wo[:, G * DH * g:G * DH * (g + 1)].T.astype(bf16)),
            "cost": cosT,
            "sints": sints,
            "masks": mask,
        })
    return in_maps


def get_nc():
    if "nc" not in _CACHE:
        _CACHE["nc"] = _build_nc()
    return _CACHE["nc"]


def kernel(x, Wq, Wk, Wv, Wo):
    from concourse.bass_utils import run_bass_kernel_spmd

    nc = get_nc()
    in_maps = build_in_maps(x, Wq, Wk, Wv, Wo)
    res = run_bass_kernel_spmd(nc, in_maps, list(range(NCORES)))
    parts = [res.results[c]["y"].astype(np.float32) for c in range(NCORES)]
    y = np.stack([
        parts[0] + parts[1] + parts[2] + parts[3],
        parts[4] + parts[5] + parts[6] + parts[7],
    ]).astype(np.float32)
    return y


# revision 49
# speedup vs baseline: 2.2573x; 1.0170x over previous
"""GQA (grouped-query attention) Trainium2 kernel, 8-core SPMD.

Sharding: TP=4 over kv-heads x DP=2 over batch  (core = b*4 + g).
Each core computes, for its batch b and kv-head g (q-heads 4g..4g+3):
  QKV projections -> RoPE -> causal softmax(QK^T)V -> partial x@Wo
entirely in transposed layout (feature dim on SBUF partitions), then the
host sums the 4 partial Wo outputs per batch (the TP all-reduce).

Single fused pipeline over the 4 query/sequence blocks of 512:
  proj(sb) -> attention(qb=sb) -> Wo-chunk(qb-1) quarters interleaved
between attention heads so the PE never drains.

Dataflow notes:
 - all matmul operands in bf16 (full PE rate, halves HBM traffic)
 - causal structure: strictly-upper k-blocks skipped; the 4 diagonal
   k-blocks use narrowed moving operands (512/384/256/128 queries) and a
   multiplicative triangular mask after exp
 - softmax runs in S^T[k,q] orientation; denominators: exp tiles are
   accumulated on the vector engine (bf16 2x) into dacc, then a single
   ones-column matmul per (head, q-block) reduces partitions on the PE
 - no max-subtraction: scores are bounded (~+-5) for this problem size
 - V^T -> V[k,dv] reorientation via DMA transpose (16-bit), not the PE
"""

import math
import sys

import numpy as np

if "/opt/trn_rl_repo" not in sys.path:
    sys.path.insert(0, "/opt/trn_rl_repo")

import ml_dtypes

B, S, D = 2, 2048, 2048
HQ, HKV, DH = 16, 4, 128
G = HQ // HKV            # q-heads per kv-head = 4
NCORES = 8
ROPE_THETA = 10000.0
SCALE = 1.0 / math.sqrt(DH)

SB = 512                 # wide column block (moving operand)
NSB = S // SB            # 4
ND = D // 128            # 16 contraction tiles
NKB = S // 128           # 16 key blocks

_CACHE = {}


def _build_nc():
    import concourse.bass as bass
    import concourse.mybir as mybir
    import concourse.tile as tile
    from concourse import bacc

    f32 = mybir.dt.float32
    bf16 = mybir.dt.bfloat16
    f32r = mybir.dt.float32r
    AF = mybir.ActivationFunctionType

    nc = bacc.Bacc(
        trn_type="TRN2", target_bir_lowering=False, debug=False,
        num_devices=NCORES,
    )

    xt_d = nc.dram_tensor("xt", [D, S], bf16, kind="ExternalInput").ap()
    wqkv_d = nc.dram_tensor("wqkv", [D, (G + 2) * DH], bf16, kind="ExternalInput").ap()
    wot_d = nc.dram_tensor("wot", [G * DH, D], bf16, kind="ExternalInput").ap()
    cos_d = nc.dram_tensor("cost", [DH, S], f32, kind="ExternalInput").ap()
    sin_d = nc.dram_tensor("sints", [DH, S], f32, kind="ExternalInput").ap()
    msk_d = nc.dram_tensor("masks", [128, SB], bf16, kind="ExternalInput").ap()
    y_d = nc.dram_tensor("y", [S, D], bf16, kind="ExternalOutput").ap()

    from contextlib import ExitStack

    with tile.TileContext(nc) as tc, ExitStack() as stack, \
            nc.allow_low_precision(reason="bf16 matmul operands"):
        persist = stack.enter_context(tc.tile_pool(name="persist", bufs=1))

        # persistent SBUF tensors (per-block tiles: no reliance on sub-tile
        # dependency tracking)
        xts = [[persist.tile([128, SB], bf16, name=f"x{s}_{i}", tag=f"x{s}_{i}")
                for i in range(ND)] for s in range(NSB)]
        wqkv = [persist.tile([128, (G + 2) * DH], bf16, name=f"w{i}", tag=f"w{i}")
                for i in range(ND)]
        wot = [persist.tile([128, D], bf16, name=f"wo{h}", tag=f"wo{h}")
               for h in range(G)]
        qrt = [[persist.tile([128, SB], bf16, name=f"q{h}_{s}", tag=f"q{h}_{s}")
                for s in range(NSB)] for h in range(G)]
        krt = [persist.tile([128, SB], bf16, name=f"k{s}", tag=f"k{s}")
               for s in range(NSB)]
        vsb = [persist.tile([128, DH], bf16, name=f"v{k}", tag=f"v{k}")
               for k in range(NKB)]
        a_t = [[persist.tile([128, SB], bf16, name=f"a{h}_{s}", tag=f"a{h}_{s}")
                for s in range(NSB)] for h in range(G)]
        cost = [persist.tile([128, SB], f32, name=f"cost{s}", tag=f"cost{s}")
                for s in range(NSB)]
        sint = [persist.tile([128, SB], f32, name=f"sint{s}", tag=f"sint{s}")
                for s in range(NSB)]
        mask = persist.tile([128, SB], bf16, name="mask", tag="mask")

        # working rings (SBUF)
        p_pool = stack.enter_context(tc.tile_pool(name="pp", bufs=4))
        dacc_pool = stack.enter_context(tc.tile_pool(name="dap", bufs=2))
        vsbw_pool = stack.enter_context(tc.tile_pool(name="vsw", bufs=2))
        rope_pool = stack.enter_context(tc.tile_pool(name="rope", bufs=2))
        fin_pool = stack.enter_context(tc.tile_pool(name="fin", bufs=2))
        yt_pool = stack.enter_context(tc.tile_pool(name="yt", bufs=2))

        # PSUM: mm(3) shared by proj/Wo + sps(3) + aps(2) = 8 banks
        mm_ps = stack.enter_context(tc.tile_pool(name="mm_ps", bufs=3, space="PSUM"))
        s_ps = stack.enter_context(tc.tile_pool(name="s_ps", bufs=3, space="PSUM"))
        a_ps = stack.enter_context(tc.tile_pool(name="a_ps", bufs=2, space="PSUM"))

        # weights + first x block, interleaved so matmul i can start early;
        # the first stripe ships its V-columns first (first PE matmul)
        nc.sync.dma_start(wqkv[0][:, (G + 1) * DH:], wqkv_d[0:128, (G + 1) * DH:])
        nc.sync.dma_start(xts[0][0][:], xt_d[0:128, 0:SB])
        nc.sync.dma_start(wqkv[0][:, 0:(G + 1) * DH], wqkv_d[0:128, 0:(G + 1) * DH])
        for i in range(1, ND):
            nc.sync.dma_start(wqkv[i][:], wqkv_d[128 * i:128 * (i + 1), :])
            nc.sync.dma_start(xts[0][i][:], xt_d[128 * i:128 * (i + 1), 0:SB])
        # first rope-table chunk right behind (gates the first rope_evict);
        # the rest follows in proj_block(0)
        nc.sync.dma_start(cost[0][:], cos_d[:, 0:SB])
        nc.sync.dma_start(sint[0][:], sin_d[:, 0:SB])
        # small tensors: needed only from attention(0) on
        nc.sync.dma_start(mask[:], msk_d[:])

        def rope_evict(ps, out_tile, sb):
            # Cross-half (rotate-half) reads live on the PSUM operand: the
            # BIR verifier only requires equal base partitions when BOTH
            # tensor_tensor inputs are in SBUF. Muls read PSUM -> DVE; the
            # final add is SBUF-only and partition-aligned -> Pool engine.
            ts_ = rope_pool.tile([128, SB], f32, name="tsin", tag="tsin")
            tcs = rope_pool.tile([128, SB], f32, name="tcos", tag="tcos")
            nc.vector.tensor_mul(ts_[0:64, :], ps[64:128, :], sint[sb][0:64, :])
            nc.vector.tensor_mul(ts_[64:128, :], ps[0:64, :], sint[sb][64:128, :])
            nc.vector.tensor_mul(tcs[:], ps[:], cost[sb][:, :])
            nc.gpsimd.tensor_add(out_tile[:], tcs[:], ts_[:])

        def proj_block(sb):
            c0 = SB * sb
            xt = xts[sb]
            KO, VO = G * DH, (G + 1) * DH  # column offsets of Wk / Wv

            def mm_group(lo, interleaved=()):
                ps = mm_ps.tile([128, SB], f32, name="pp", tag="pp")
                for i in range(ND):
                    nc.tensor.matmul(
                        ps[:], wqkv[i][:, lo:lo + DH], xt[i][:],
                        start=(i == 0), stop=(i == ND - 1),
                        skip_group_check=bool(interleaved))
                return ps

            if sb == 0:
                # x0 tiles arrive at DMA pace: interleave all 6 psum groups
                # by contraction index so the PE rides the DMA wave,
                # borrowing the (not yet used) attention PSUM banks
                pools = [mm_ps, mm_ps, mm_ps, s_ps, s_ps, a_ps]
                tags = ["pp", "pp", "pp", "sps", "sps", "aps"]
                offsets = [VO, KO, 0, DH, 2 * DH, 3 * DH]
                pss = [pool.tile([128, SB], f32, name="pp", tag=t)
                       for pool, t in zip(pools, tags)]
                for i in range(ND):
                    for ps, lo in zip(pss, offsets):
                        nc.tensor.matmul(
                            ps[:], wqkv[i][:, lo:lo + DH], xt[i][:],
                            start=(i == 0), stop=(i == ND - 1),
                            skip_group_check=True)
                vps, kps, qps0, qps1, qps2, qps3 = pss
                rope_evict(kps, krt[sb], sb)
                v_sb = vsbw_pool.tile([128, SB], bf16, name="vsb", tag="vsb")
                nc.scalar.copy(v_sb[:], vps[:])
                rope_evict(qps0, qrt[0][sb], sb)
                rope_evict(qps1, qrt[1][sb], sb)
                rope_evict(qps2, qrt[2][sb], sb)
                rope_evict(qps3, qrt[3][sb], sb)
            else:
                vps = mm_group(VO)
                kps = mm_group(KO)
                v_sb = vsbw_pool.tile([128, SB], bf16, name="vsb", tag="vsb")
                nc.scalar.copy(v_sb[:], vps[:])
                qps0 = mm_group(0)
                rope_evict(kps, krt[sb], sb)
                qps1 = mm_group(DH)
                rope_evict(qps0, qrt[0][sb], sb)
                qps2 = mm_group(2 * DH)
                rope_evict(qps1, qrt[1][sb], sb)
                qps3 = mm_group(3 * DH)
                rope_evict(qps2, qrt[2][sb], sb)
                rope_evict(qps3, qrt[3][sb], sb)

            # DMA schedule: V transpose, tables/weights for upcoming phases,
            # next x block (order = consumption order on the sync queue)
            for c in range(SB // 128):
                nc.sync.dma_start_transpose(
                    vsb[4 * sb + c][:], v_sb[:, 128 * c:128 * (c + 1)])
            if sb + 1 < NSB:
                for i in range(ND):
                    nc.sync.dma_start(
                        xts[sb + 1][i][:],
                        xt_d[128 * i:128 * (i + 1), SB * (sb + 1):SB * (sb + 2)])
            if sb == 0:
                for s in range(1, NSB):
                    nc.sync.dma_start(cost[s][:], cos_d[:, SB * s:SB * (s + 1)])
                    nc.sync.dma_start(sint[s][:], sin_d[:, SB * s:SB * (s + 1)])
            if sb == 1:
                for h in range(G):
                    nc.sync.dma_start(wot[h][:], wot_d[128 * h:128 * (h + 1), :])

        # deferred finalize steps, drained one per attention block
        fin_steps = []

        def drain_one():
            if fin_steps:
                step = fin_steps.pop(0)
                if step is not None:
                    step()

        def drain_all():
            while fin_steps:
                step = fin_steps.pop(0)
                if step is not None:
                    step()

        def finalize_lazy(h, qb, aps, dacc, daccp):
            """Denominator partition-reduce (Pool) -> reciprocal -> normalize,
            as deferred steps.

            Each step is drained with spacing so cross-engine latency
            (DVE->Pool->DVE) never stalls the PE. partition_all_reduce leaves
            the sum on ALL partitions, so no broadcast step is needed.
            """
            from concourse import bass_isa
            st = {}

            def s0():
                nc.gpsimd.tensor_add(daccp[:], daccp[:], dacc[:])

            def s1():
                dall = fin_pool.tile([128, SB], f32, name="dall", tag="dall")
                nc.gpsimd.partition_all_reduce(
                    dall[:], daccp[:], channels=128,
                    reduce_op=bass_isa.ReduceOp.add)
                st["dall"] = dall

            def s2():
                rbc = fin_pool.tile([128, SB], f32, name="rbc", tag="rbc")
                nc.vector.reciprocal(rbc[:], st["dall"][:])
                st["rbc"] = rbc

            def s3():
                nc.vector.tensor_mul(a_t[h][qb][:], aps[:], st["rbc"][:])

            fin_steps.extend([s0, s1, None, s2, None, s3])

        def attn_head(h, qb, defer_finalize=False):
            """scores -> exp -> (mask) -> dacc accumulate -> PV accumulate.

            PE emission has one-block lookahead: scores(kb+1) before PV(kb).
            """
            nkb = 4 * qb + 4
            aps = a_ps.tile([128, SB], f32, name="aps", tag="aps")
            # two partial denominator accumulators: even k-blocks on DVE,
            # odd on Pool (both SBUF-only, legal for GPSIMD); combined at
            # the end on Pool
            dacc = dacc_pool.tile([128, SB], bf16, name="dacc", tag="dacc")
            daccp = dacc_pool.tile([128, SB], bf16, name="daccp", tag="daccp")
            pend = []  # (kb, p, qoff, w), lookahead-2 queue
            for kb in range(nkb):
                j = kb - 4 * qb
                qoff = 128 * j if j > 0 else 0
                w = SB - qoff
                sps = s_ps.tile([128, SB], f32, name="sps", tag="sps")
                nc.tensor.matmul(
                    sps[:, 0:w], krt[kb // 4][:, 128 * (kb % 4):128 * (kb % 4 + 1)],
                    qrt[h][qb][:, qoff:SB],
                    start=True, stop=True, skip_group_check=True)
                p = p_pool.tile([128, SB], bf16, name="p", tag="p")
                nc.scalar.activation(p[:, 0:w], sps[:, 0:w], AF.Exp, scale=SCALE)
                if j >= 0:
                    nc.vector.tensor_mul(p[:, 0:w], p[:, 0:w], mask[:, 0:w])
                if defer_finalize:
                    # single DVE accumulator: drops the combine step from the
                    # latency-critical last-head finalize chain
                    eng, acc = nc.vector, dacc
                else:
                    eng, acc = ((nc.vector, dacc) if kb % 2 == 0
                                else (nc.gpsimd, daccp))
                if kb < 2:
                    if qoff:
                        eng.memset(acc[:, 0:qoff], 0.0)
                    eng.tensor_copy(acc[:, qoff:SB], p[:, 0:w])
                else:
                    eng.tensor_add(acc[:, qoff:SB], acc[:, qoff:SB], p[:, 0:w])
                if len(pend) == 2:
                    pkb, pp, pqoff, pw = pend.pop(0)
                    nc.tensor.matmul(
                        aps[:, pqoff:SB], vsb[pkb][:], pp[:, 0:pw],
                        start=(pkb == 0), stop=False, skip_group_check=True)
                pend.append((kb, p, qoff, w))
                drain_one()
            while pend:
                pkb, pp, pqoff, pw = pend.pop(0)
                nc.tensor.matmul(
                    aps[:, pqoff:SB], vsb[pkb][:], pp[:, 0:pw],
                    start=(pkb == 0), stop=(not pend), skip_group_check=True)
            if defer_finalize:
                return aps, dacc, daccp
            finalize_lazy(h, qb, aps, dacc, daccp)
            return None

        def wo_quarter(qb, c, final=False):
            """y rows [128c'..] for query block qb, quarter c: 4 eb psums.

            Evictions go to the Pool engine (spare capacity); the final
            quarter evicts on Act and DMAs per-eb to shorten the tail.
            """
            yt = yt_pool.tile([128, D], bf16, name="yt", tag="yt")
            sb128 = 4 * qb + c
            for eb in range(NSB):
                yp = mm_ps.tile([128, SB], f32, name="pp", tag="pp")
                for h in range(G):
                    nc.tensor.matmul(
                        yp[:], a_t[h][qb][:, 128 * c:128 * (c + 1)],
                        wot[h][:, SB * eb:SB * (eb + 1)],
                        start=(h == 0), stop=(h == G - 1))
                if final:
                    # per-eb evict+DMA pipeline to shorten the tail
                    nc.scalar.copy(yt[:, SB * eb:SB * (eb + 1)], yp[:])
                    nc.sync.dma_start(
                        y_d[128 * sb128:128 * (sb128 + 1),
                            SB * eb:SB * (eb + 1)],
                        yt[:, SB * eb:SB * (eb + 1)])
                elif eb % 2 == 0:
                    nc.scalar.copy(yt[:, SB * eb:SB * (eb + 1)], yp[:])
                else:
                    nc.vector.tensor_copy(yt[:, SB * eb:SB * (eb + 1)], yp[:])
            if not final:
                nc.sync.dma_start(
                    y_d[128 * sb128:128 * (sb128 + 1), :], yt[:])

        last = None
        for sb in range(NSB):
            proj_block(sb)
            drain_all()  # a_t[*][sb-1] must be written before wo_quarter reads
            for h in range(G):
                if sb > 0:
                    wo_quarter(sb - 1, h)
                last = attn_head(h, sb,
                                 defer_finalize=(sb == NSB - 1 and h == G - 1))
        drain_all()

        # last head's finalize is latency-critical (it gates the final Wo
        # chunk): run it at 128-column granularity, pipelined against the
        # final Wo quarters
        from concourse import bass_isa
        aps, dacc, daccp = last
        qb = NSB - 1
        dall = fin_pool.tile([128, SB], f32, name="dall", tag="dall")
        for c in range(G):
            cs = slice(128 * c, 128 * (c + 1))
            nc.gpsimd.partition_all_reduce(
                dall[:, cs], dacc[:, cs], channels=128,
                reduce_op=bass_isa.ReduceOp.add)
            rbc = fin_pool.tile([128, 128], f32, name="rbcc", tag="rbcc")
            nc.vector.reciprocal(rbc[:], dall[:, cs])
            nc.vector.tensor_mul(a_t[G - 1][qb][:, cs], aps[:, cs], rbc[:])
            wo_quarter(qb, c, final=(c == G - 1))

    nc.compile()
    return nc


def _rope_tables():
    inv = 1.0 / (ROPE_THETA ** (np.arange(0, DH, 2, dtype=np.float64) / DH))
    pos = np.arange(S, dtype=np.float64)
    theta = np.concatenate([np.outer(pos, inv)] * 2, axis=1)  # [S, DH]
    cosT = np.cos(theta).T.astype(np.float32)                 # [DH, S]
    sinT = np.sin(theta).T.astype(np.float32)
    sints = np.concatenate([-sinT[:64], sinT[64:]], axis=0)
    return np.ascontiguousarray(cosT), np.ascontiguousarray(sints)


def build_in_maps(x, Wq, Wk, Wv, Wo):
    bf16 = ml_dtypes.bfloat16
    x = np.asarray(x, np.float32)
    Wq = np.asarray(Wq, np.float32)
    Wk = np.asarray(Wk, np.float32)
    Wv = np.asarray(Wv, np.float32)
    Wo = np.asarray(Wo, np.float32)
    cosT, sints = _rope_tables()
    r_ = np.arange(128)[:, None]
    c_ = np.arange(SB)[None, :]
    mask = (c_ >= r_).astype(np.float32).astype(bf16)
    xt_b = [np.ascontiguousarray(x[b].T.astype(bf16)) for b in range(B)]
    in_maps = []
    for core in range(NCORES):
        b, g = divmod(core, HKV)
        wqkv = np.concatenate([
            Wq[G * DH * g:G * DH * (g + 1)].T,
            Wk[DH * g:DH * (g + 1)].T,
            Wv[DH * g:DH * (g + 1)].T,
        ], axis=1).astype(bf16)
        in_maps.append({
            "xt": xt_b[b],
            "wqkv": np.ascontiguousarray(wqkv),
            "wot": np.ascontiguousarray(
                Wo[:, G * DH * g:G * DH * (g + 1)].T.astype(bf16)),
            "cost": cosT,
            "sints": sints,
            "masks": mask,
        })
    return in_maps


def get_nc():
    if "nc" not in _CACHE:
        _CACHE["nc"] = _build_nc()
    return _CACHE["nc"]


def kernel(x, Wq, Wk, Wv, Wo):
    from concourse.bass_utils import run_bass_kernel_spmd

    nc = get_nc()
    in_maps = build_in_maps(x, Wq, Wk, Wv, Wo)
    res = run_bass_kernel_spmd(nc, in_maps, list(range(NCORES)))
    parts = [res.results[c]["y"].astype(np.float32) for c in range(NCORES)]
    y = np.stack([
        parts[0] + parts[1] + parts[2] + parts[3],
        parts[4] + parts[5] + parts[6] + parts[7],
    ]).astype(np.float32)
    return y
